# revision 1
# baseline (speedup 1.0000x reference)
import sys
sys.path.insert(0, "/opt/trn_rl_repo")
import time as _time
import numpy as np

N = 100000
E = 800000
D = 128
P = 8
NSH = 12500          # nodes per core
NSH_PAD = 12544      # 98 * 128
NFULL = P * NSH_PAD  # 100352 rows in allgathered x
ALPHA = 0.5
CALL = 896           # idxs per call (56 idx cols per call: ring-safe)
COLS = CALL // 16
# gather chunks over x_full rows (int16 idx limit 32767): chunk c = rows
# [32768c, 32768c+32768); chunk 3 is short (2048 rows)
NCHUNK = 4
CHUNK_ROWS = (32768, 32768, 32768, NFULL - 3 * 32768)
# per-direction per-chunk call capacity (fixed program shape; ~20% margin
# over the expected rank-grouped call count for E/P random edges)
CAPS = (50, 50, 50, 8)
NC_DIR = sum(CAPS)          # calls per direction
CHUNK_BASE = (0, CAPS[0], CAPS[0] + CAPS[1], CAPS[0] + CAPS[1] + CAPS[2])

_verbose = True


def _log(tag, t0):
    if _verbose:
        now = _time.perf_counter()
        print(f"[kernel-timing] {tag}: {now - t0:.3f}s", file=sys.stderr)
    return _time.perf_counter()


# ---------------------------------------------------------------- planner ---

def _plan_direction(gat, seg, caps):
    """Vectorized duplicate-free call plan for one direction.

    gat: global gather node per edge; seg: global segment (scatter) node.
    Returns (gflat, sflat) int16 arrays [P, NC_DIR*CALL] filled with -1 pads,
    plus per-(core,chunk) needed call counts [P, NCHUNK].
    Within one call every scatter target is unique (edges grouped by
    round-robin rank within their (chunk, segment)), so no same-row RMW
    races inside a call.
    """
    seg = seg.astype(np.int32)
    gat = gat.astype(np.int32)
    core = seg // NSH
    loc = seg - core * NSH
    q = gat // NSH
    row = q * NSH_PAD + (gat - q * NSH)
    chunk = row >> 15
    lidx = row & 32767

    cc = core * NCHUNK + chunk                 # 0..31
    # sort by (core, chunk, seg); rank = run position within equal seg
    k1 = cc * (1 << 17) + seg                  # < 2^22, int32 radix sort
    o1 = np.argsort(k1, kind="stable")
    k1s = k1[o1]
    first = np.empty(E, bool)
    first[0] = True
    np.not_equal(k1s[1:], k1s[:-1], out=first[1:])
    ar = np.arange(E, dtype=np.int64)
    idx_first = np.maximum.accumulate(np.where(first, ar, 0))
    rank = (ar - idx_first).astype(np.int32)
    # sort by (core, chunk, rank) stable -> final edge order
    k2 = cc[o1] * E + rank                     # < 2^25, int32 radix sort
    o2 = np.argsort(k2, kind="stable")
    k2s = k2[o2]
    of = o1[o2]
    # position within each (core, chunk, rank) run
    first2 = np.empty(E, bool)
    first2[0] = True
    np.not_equal(k2s[1:], k2s[:-1], out=first2[1:])
    idx_first2 = np.maximum.accumulate(np.where(first2, ar, 0))
    posr = ar - idx_first2
    call_in_run = posr // CALL
    slot = posr - call_in_run * CALL
    # per-run call counts -> per-(core,chunk) cumulative call base per rank
    run_starts = np.flatnonzero(first2)
    run_lens = np.diff(np.r_[run_starts, E])
    run_calls = (run_lens + CALL - 1) // CALL
    run_cc = cc[of[run_starts]]
    # cumulative calls of earlier ranks within same (core,chunk)
    csum = np.cumsum(run_calls) - run_calls
    cc_first_run = np.empty(run_cc.size, bool)
    cc_first_run[0] = True
    np.not_equal(run_cc[1:], run_cc[:-1], out=cc_first_run[1:])
    arr_r = np.arange(run_cc.size)
    idx_first_cc = np.maximum.accumulate(np.where(cc_first_run, arr_r, 0))
    run_base = csum - csum[idx_first_cc]
    needed = np.zeros((P, NCHUNK), np.int64)
    last_of_cc = np.r_[cc_first_run[1:], True]
    needed[run_cc[last_of_cc] // NCHUNK, run_cc[last_of_cc] % NCHUNK] = \
        (run_base + run_calls)[last_of_cc]
    if np.any(needed > np.asarray(caps)[None, :]):
        return None, None, needed
    # flat destination index
    call_idx = run_base[np.cumsum(first2) - 1] + call_in_run
    ch_base = np.asarray(
        [0, caps[0], caps[0] + caps[1], caps[0] + caps[1] + caps[2]])
    nc_dir = int(sum(caps))
    core_f = core[of]
    chunk_f = chunk[of]
    flat = ((core_f * nc_dir + ch_base[chunk_f] + call_idx) * CALL + slot)
    # pads gather row 0 of their chunk and scatter into agg row NSH (a
    # discarded pad row); same-row pad adds may race but are never read
    gflat = np.zeros(P * nc_dir * CALL, np.int16)
    sflat = np.full(P * nc_dir * CALL, NSH, np.int16)
    gflat[flat] = lidx[of].astype(np.int16)
    sflat[flat] = loc[of].astype(np.int16)
    return (gflat.reshape(P, nc_dir * CALL),
            sflat.reshape(P, nc_dir * CALL), needed)


def _wrap16(a):
    # idx i -> [i % 16, i // 16]
    return np.ascontiguousarray(a.reshape(-1, 16).T)


# ------------------------------------------------------------- device prog --

def _build_program(caps):
    from concourse import bacc, tile, mybir, library_config

    f32 = mybir.dt.float32
    f16 = mybir.dt.float16
    i16 = mybir.dt.int16
    nc = bacc.Bacc("TRN2", target_bir_lowering=False, debug=False,
                   num_swdge_queues=3, num_devices=P)

    nc_dir = int(sum(caps))
    xs = nc.dram_tensor("x_sh", [NSH_PAD, D], f16, kind="ExternalInput")
    # all four idx streams packed in one tensor (rows: 16 per stream, order
    # gidx_in, sidx_in, gidx_out, sidx_out); all small f32 constants packed
    # in one [128, 709] tensor (cols: inv_in 0:98, inv_out 98:196,
    # W_self 196:324, W1 324:452, W2 452:580, b 580:581, ident 581:709)
    idx_all = nc.dram_tensor("idx_all", [64, nc_dir * COLS], i16,
                             kind="ExternalInput")
    consts = nc.dram_tensor("consts", [128, 709], f32, kind="ExternalInput")
    x_bounce = nc.dram_tensor("x_bounce", [NSH_PAD, D], f16)
    # NOTE: addr_space="Shared" for the AllGather output desyncs the mesh
    # under the axon PJRT path; plain DRAM works (slower CC but tiny anyway).
    x_full = nc.dram_tensor("x_full", [NFULL, D], f16)
    agg_in = nc.dram_tensor("agg_in", [NSH_PAD, D], f32)
    agg_out = nc.dram_tensor("agg_out", [NSH_PAD, D], f32)
    out = nc.dram_tensor("out", [NSH_PAD, D], f16, kind="ExternalOutput")

    with tile.TileContext(nc) as tc:
        nc.gpsimd.load_library(library_config.mlp)
        with tc.tile_pool(name="const", bufs=1) as cp, \
             tc.tile_pool(name="gt", bufs=2) as gp, \
             tc.tile_pool(name="ep", bufs=3) as ep, \
             tc.tile_pool(name="ps", bufs=2, space="PSUM") as pp:
            # shard -> bounce -> allgather into x_full
            nc.sync.dma_start(x_bounce[:], xs[:])
            nc.gpsimd.collective_compute(
                "AllGather", mybir.AluOpType.bypass,
                replica_groups=[list(range(P))],
                ins=[x_bounce.ap().opt()],
                outs=[x_full.ap().opt()],
            )

            # index tiles: load 16 rows, replicate to 128 partitions on device
            idx_tiles = {}
            for j, nm in enumerate(("gi", "si", "go", "so")):
                t = cp.tile([128, nc_dir * COLS], i16, tag=f"idx_{nm}")
                nc.sync.dma_start(t[0:16, :], idx_all[j * 16:(j + 1) * 16, :])
                nc.sync.dma_start(t[16:32, :], idx_all[j * 16:(j + 1) * 16, :])
                nc.sync.dma_start(t[32:64, :], t[0:32, :])
                nc.sync.dma_start(t[64:128, :], t[0:64, :])
                idx_tiles[nm] = t
            cs = cp.tile([128, 709], f32)
            nc.sync.dma_start(cs[:], consts[:])
            C_IVI, C_IVO, C_WS, C_W1, C_W2, C_B, C_ID = (
                0, 98, 196, 324, 452, 580, 581)

            # zero agg buffers from an SBUF zero tile
            zt = cp.tile([128, NSH_PAD], f32)
            nc.vector.memset(zt[:], 0.0)
            for t in range(98):
                nc.sync.dma_start(agg_in[t * 128:(t + 1) * 128, :],
                                  zt[:, t * 128:(t + 1) * 128])
                nc.sync.dma_start(agg_out[t * 128:(t + 1) * 128, :],
                                  zt[:, t * 128:(t + 1) * 128])

            # gather + convert + scatter chains; a dummy reader of the agg
            # buffer between consecutive same-buffer scatters forces each
            # scatter's DMA to complete before the next starts (cross-call
            # same-row RMWs on different DMA engines would otherwise race).
            def emit_call(k, c, gkey, skey, agg, dirtag):
                t16 = gp.tile([128, CALL // 128, D], f16, tag="g16")
                if c < 3:
                    src = x_full[c * 32768:(c + 1) * 32768, :]
                else:
                    src = x_full[3 * 32768:NFULL, :]
                nc.gpsimd.dma_gather(
                    t16[:], src,
                    idx_tiles[gkey][:, k * COLS:(k + 1) * COLS],
                    CALL, CALL, D, queue_num=0)
                t32 = gp.tile([128, CALL // 128, D], f32, tag="g32")
                nc.vector.tensor_copy(t32[:], t16[:])
                dr = gp.tile([1, 64], f32, tag=f"dummy{dirtag}")
                nc.sync.dma_start(dr[:], agg[0:1, 0:64])
                nc.gpsimd.dma_scatter_add(
                    agg[:], t32[:],
                    idx_tiles[skey][:, k * COLS:(k + 1) * COLS],
                    CALL, CALL, D, queue_num=1 if dirtag == "i" else 2)

            # interleave the 8 (direction, chunk) streams round-robin so that
            # consecutive same-buffer scatters sharing a dst row are far
            # apart in time (adjacent same-stream calls are dst-disjoint
            # splits of one rank group or consecutive ranks)
            ch_base = (0, caps[0], caps[0] + caps[1], caps[0] + caps[1] + caps[2])
            for r in range(max(caps)):
                for c in range(NCHUNK):
                    if r < caps[c]:
                        emit_call(ch_base[c] + r, c, "gi", "si", agg_in, "i")
                        emit_call(ch_base[c] + r, c, "go", "so", agg_out, "o")

            # epilogue per 128-node tile
            for t in range(98):
                ai = ep.tile([128, D], f32, tag="ai")
                ao = ep.tile([128, D], f32, tag="ao")
                nc.sync.dma_start(ai[:], agg_in[t * 128:(t + 1) * 128, :])
                nc.sync.dma_start(ao[:], agg_out[t * 128:(t + 1) * 128, :])
                # scale by inv degree (per-partition scalar)
                nc.vector.tensor_scalar(ai[:], ai[:],
                                        cs[:, C_IVI + t:C_IVI + t + 1], None,
                                        mybir.AluOpType.mult)
                nc.vector.tensor_scalar(ao[:], ao[:],
                                        cs[:, C_IVO + t:C_IVO + t + 1], None,
                                        mybir.AluOpType.mult)
                # own-shard x tile: load f16, convert to f32
                xt16 = ep.tile([128, D], f16, tag="xt16")
                nc.sync.dma_start(xt16[:], xs[t * 128:(t + 1) * 128, :])
                xt = ep.tile([128, D], f32, tag="xt")
                nc.vector.tensor_copy(xt[:], xt16[:])
                # transpose all three activations
                pt = pp.tile([128, D], f32, tag="pt")
                nc.tensor.matmul(pt[:], ai[:], cs[:, C_ID:C_ID + D],
                                 start=True, stop=True, is_transpose=True)
                aiT = ep.tile([128, D], f32, tag="aiT")
                nc.vector.tensor_copy(aiT[:], pt[:])
                pt2 = pp.tile([128, D], f32, tag="pt")
                nc.tensor.matmul(pt2[:], ao[:], cs[:, C_ID:C_ID + D],
                                 start=True, stop=True, is_transpose=True)
                aoT = ep.tile([128, D], f32, tag="aoT")
                nc.vector.tensor_copy(aoT[:], pt2[:])
                pt3 = pp.tile([128, D], f32, tag="pt")
                nc.tensor.matmul(pt3[:], xt[:], cs[:, C_ID:C_ID + D],
                                 start=True, stop=True, is_transpose=True)
                xtT = ep.tile([128, D], f32, tag="xtT")
                nc.vector.tensor_copy(xtT[:], pt3[:])
                # y = W_self.T @ xT + W1.T @ aiT + W2.T @ aoT   [feat_out, nodes]
                y = pp.tile([128, 128], f32, tag="y")
                nc.tensor.matmul(y[:], cs[:, C_WS:C_WS + D], xtT[:],
                                 start=True, stop=False)
                nc.tensor.matmul(y[:], cs[:, C_W1:C_W1 + D], aiT[:],
                                 start=False, stop=False)
                nc.tensor.matmul(y[:], cs[:, C_W2:C_W2 + D], aoT[:],
                                 start=False, stop=True)
                ysb = ep.tile([128, 128], f32, tag="ysb")
                nc.vector.tensor_scalar(ysb[:], y[:], cs[:, C_B:C_B + 1], None,
                                        mybir.AluOpType.add)
                # transpose back to [nodes, feat], convert to f16
                po = pp.tile([128, 128], f32, tag="po")
                nc.tensor.matmul(po[:], ysb[:], cs[:, C_ID:C_ID + D],
                                 start=True, stop=True, is_transpose=True)
                osb = ep.tile([128, 128], f16, tag="osb")
                nc.vector.tensor_copy(osb[:], po[:])
                nc.sync.dma_start(out[t * 128:(t + 1) * 128, :], osb[:])

    nc.compile()
    return nc


# --------------------------------------------------------------- AOT setup --

_AOT = {}


def _aot_compile(caps):
    t0 = _time.perf_counter()
    import jax
    from jax.sharding import Mesh, PartitionSpec, NamedSharding
    from jax.experimental.shard_map import shard_map
    from concourse import bass2jax, mybir

    nc = _build_program(caps)
    t0 = _log("aot: build+bass-compile", t0)

    bass2jax.install_neuronx_cc_hook()
    partition_name = nc.partition_id_tensor.name if nc.partition_id_tensor else None
    in_names, out_names, out_avals, zero_outs = [], [], [], []
    for alloc in nc.m.functions[0].allocations:
        if not isinstance(alloc, mybir.MemoryLocationSet):
            continue
        name = alloc.memorylocations[0].name
        if alloc.kind == "ExternalInput":
            if name != partition_name:
                in_names.append(name)
        elif alloc.kind == "ExternalOutput":
            shape = tuple(alloc.tensor_shape)
            dtype = mybir.dt.np(alloc.dtype)
            out_names.append(name)
            out_avals.append(jax.core.ShapedArray(shape, dtype))
            zero_outs.append(np.zeros(shape, dtype))
    n_params = len(in_names)
    n_outs = len(out_avals)
    in_names_full = in_names + out_names + ([partition_name] if partition_name else [])

    def _body(*args):
        operands = list(args)
        if partition_name is not None:
            operands.append(bass2jax.partition_id_tensor())
        outs = bass2jax._bass_exec_p.bind(
            *operands,
            out_avals=tuple(out_avals),
            in_names=tuple(in_names_full),
            out_names=tuple(out_names),
            lowering_input_output_aliases=(),
            sim_require_finite=True,
            sim_require_nnan=True,
            nc=nc,
        )
        return tuple(outs)

    devices = jax.devices()[:P]
    mesh = Mesh(np.asarray(devices), ("core",))
    in_specs = (PartitionSpec("core"),) * (n_params + n_outs)
    out_specs = (PartitionSpec("core"),) * n_outs
    donate = tuple(range(n_params, n_params + n_outs))
    sharded = jax.jit(
        shard_map(_body, mesh=mesh, in_specs=in_specs, out_specs=out_specs,
                  check_rep=False),
        donate_argnums=donate, keep_unused=True)

    # abstract shapes: per-core input shapes concatenated over cores on axis 0
    shape_of = {}
    for alloc in nc.m.functions[0].allocations:
        if isinstance(alloc, mybir.MemoryLocationSet) and alloc.kind == "ExternalInput":
            shape_of[alloc.memorylocations[0].name] = (
                tuple(alloc.tensor_shape), mybir.dt.np(alloc.dtype))
    abstract = []
    for name in in_names:
        shp, dt = shape_of[name]
        abstract.append(jax.ShapeDtypeStruct((P * shp[0],) + shp[1:], dt))
    for z in zero_outs:
        abstract.append(jax.ShapeDtypeStruct((P * z.shape[0],) + z.shape[1:], z.dtype))
    compiled = sharded.lower(*abstract).compile()
    t0 = _log("aot: neff-compile", t0)

    sharding = NamedSharding(mesh, PartitionSpec("core"))
    _AOT.update(dict(
        caps=tuple(caps), nc=nc, compiled=compiled, in_names=in_names,
        out_names=out_names, zero_outs=zero_outs, sharding=sharding,
        dz=None, jax=jax))
    _make_zeros()
    _log("aot: zero-put", t0)


def _make_zeros():
    jax = _AOT["jax"]
    _AOT["dz"] = [
        jax.device_put(
            np.zeros((P * z.shape[0],) + z.shape[1:], z.dtype), _AOT["sharding"])
        for z in _AOT["zero_outs"]]
    jax.block_until_ready(_AOT["dz"])


import os
if not os.environ.get("KERNEL_NO_AOT"):
    try:
        _aot_compile(CAPS)
    except Exception as _e:  # pragma: no cover - fall through to lazy compile
        print(f"[kernel] AOT compile failed ({_e!r}); will compile lazily",
              file=sys.stderr)
        _AOT.clear()


# ------------------------------------------------------------------ kernel --

def kernel(x, W_self, b_self, W_s2d, b_s2d, W_d2s, b_d2s, edge_index):
    t0 = _time.perf_counter()
    x = np.ascontiguousarray(x, np.float32)
    W_self = np.asarray(W_self, np.float32)
    b_self = np.asarray(b_self, np.float32)
    W_s2d = np.asarray(W_s2d, np.float32)
    b_s2d = np.asarray(b_s2d, np.float32)
    W_d2s = np.asarray(W_d2s, np.float32)
    b_d2s = np.asarray(b_d2s, np.float32)
    src = np.asarray(edge_index[0], np.int64)
    dst = np.asarray(edge_index[1], np.int64)

    jax = _AOT.get("jax")
    if jax is None:
        import jax  # lazy path
    sharding = _AOT.get("sharding")

    # x conversion first (cheap, serial), then planner threads overlap with
    # the x upload and the consts packing on the main thread
    from concurrent.futures import ThreadPoolExecutor
    caps = _AOT.get("caps", CAPS)
    nc_dir = int(sum(caps))
    x16 = np.zeros((NFULL, D), np.float16)
    x16.reshape(P, NSH_PAD, D)[:, :NSH] = x.reshape(P, NSH, D).astype(np.float16)

    def _pack_idx(v, out):
        # idx i -> [i % 16, i // 16] per core
        out[:] = v.reshape(P, nc_dir * COLS, 16).transpose(0, 2, 1)

    def _build_consts():
        deg_in = np.bincount(dst, minlength=N).astype(np.float32)
        deg_out = np.bincount(src, minlength=N).astype(np.float32)
        con = np.zeros((P, 128, 709), np.float32)
        for col, v in ((0, 1.0 / np.maximum(deg_in, 1.0)),
                       (98, 1.0 / np.maximum(deg_out, 1.0))):
            a = np.zeros((P, NSH_PAD), np.float32)
            a[:, :NSH] = v.reshape(P, NSH)
            con[:, :, col:col + 98] = a.reshape(P, 98, 128).transpose(0, 2, 1)
        con[:, :, 196:324] = W_self
        con[:, :, 324:452] = (1.0 - ALPHA) * W_s2d
        con[:, :, 452:580] = ALPHA * W_d2s
        b_tot = b_self + (1.0 - ALPHA) * b_s2d + ALPHA * b_d2s
        con[:, :, 580] = b_tot
        con[:, :, 581:709] = np.eye(D, dtype=np.float32)
        return con.reshape(P * 128, 709)

    def _plan_all(caps_, idx_host, pool):
        f_in = pool.submit(_plan_direction, src, dst, caps_)
        f_out = pool.submit(_plan_direction, dst, src, caps_)
        gi, si, need_i = f_in.result()
        go, so, need_o = f_out.result()
        if gi is None or go is None:
            return np.maximum(need_i, need_o)
        nd = int(sum(caps_))
        for j, arr in enumerate((gi, si, go, so)):
            idx_host.reshape(P, 64, nd * COLS)[:, j * 16:(j + 1) * 16] = \
                arr.reshape(P, nd * COLS, 16).transpose(0, 2, 1)
        return None

    with ThreadPoolExecutor(3) as ex:
        idx_host = np.empty((P * 64, nc_dir * COLS), np.int16)
        f_plan = ex.submit(_plan_all, caps, idx_host, ex)
        # main: x upload + consts pack/upload overlap with planning
        dev_x = jax.device_put(x16, sharding) if sharding is not None else None
        con = _build_consts()
        dev_con = (jax.device_put(con, sharding)
                   if sharding is not None else None)
        need = f_plan.result()

    if need is not None:
        # capacity exceeded: recompile with room and redo plans
        caps = tuple(int(v)
                     for v in np.maximum(np.asarray(caps), need.max(0) + 2))
        print(f"[kernel] capacity exceeded; recompiling with caps={caps}",
              file=sys.stderr)
        _aot_compile(caps)
        sharding = _AOT["sharding"]
        jax = _AOT["jax"]
        nc_dir = int(sum(caps))
        dev_x = jax.device_put(x16, sharding)
        dev_con = jax.device_put(con, sharding)
        idx_host = np.empty((P * 64, nc_dir * COLS), np.int16)
        with ThreadPoolExecutor(3) as ex:
            assert _plan_all(caps, idx_host, ex) is None
    t0 = _log("plan+pack+put", t0)

    if not _AOT:
        _aot_compile(caps)
        jax = _AOT["jax"]
        sharding = _AOT["sharding"]
        dev_x = jax.device_put(x16, sharding)
        dev_con = jax.device_put(con, sharding)
    compiled = _AOT["compiled"]
    dev_idx = jax.device_put(idx_host, sharding)

    by_name = {"x_sh": dev_x, "idx_all": dev_idx, "consts": dev_con}
    dev_args = [by_name[name] for name in _AOT["in_names"]]
    if _AOT["dz"] is None:
        _make_zeros()
    dz = _AOT["dz"]
    _AOT["dz"] = None  # consumed by donation
    jax.block_until_ready(dev_args)
    t0 = _log("upload", t0)

    try:
        outs = compiled(*dev_args, *dz)
        jax.block_until_ready(outs)
    except Exception as e:  # transient device failure: one retry
        print(f"[kernel] exec failed ({e!r}); retrying once", file=sys.stderr)
        _make_zeros()
        dz = _AOT["dz"]
        _AOT["dz"] = None
        outs = compiled(*dev_args, *dz)
        jax.block_until_ready(outs)
    t0 = _log("exec", t0)

    # fetch the 8 output shards in parallel and convert per shard
    res = np.empty((N, D), np.float32)
    res_v = res.reshape(P, NSH, D)
    shards = list(outs[0].addressable_shards)

    def _fetch(sh):
        c = sh.index[0].start // NSH_PAD if sh.index[0].start else 0
        res_v[c] = np.asarray(sh.data)[:NSH].astype(np.float32)

    with ThreadPoolExecutor(P) as ex:
        list(ex.map(_fetch, shards))
    _log("fetch+convert", t0)
    return res



# revision 2
# speedup vs baseline: 1.7081x; 1.7081x over previous
import sys
sys.path.insert(0, "/opt/trn_rl_repo")
import os
import time as _time
import numpy as np

# DirSageConv on 8 TRN2 cores.
# Device algorithm: int8 x upload -> on-device dequant -> AllGather f16 ->
# per-dst-tile gather of neighbor rows (dst-sorted, chunked by x_full row
# range for int16 gather indices) -> one-hot matmul segment-sum in PSUM ->
# fused epilogue (3 GEMMs + transposes + per-node mean scaling) -> int8
# row-quantized output fetched and dequantized on host.
N = 100000
E = 800000
D = 128
P = 8
NSH = 12500
NSH_PAD = 12544      # 98 * 128
NT = 98              # node tiles per core
NFULL = P * NSH_PAD  # 100352 rows in allgathered x
ALPHA = 0.5
SUP = 7              # tiles per gather super-group
NSUP = 14
CAPS_SLOT = (512, 512, 512, 128)   # slots per (tile, chunk), 128-aligned
CUMCAP = (0, 512, 1024, 1536)
NB_C = (4, 4, 4, 1)                # 128-blocks per (tile, chunk)
NBLK_TILE = 13
TSLOT = 1664
S = NT * TSLOT        # 163072 slots per direction per core
SUP_SLOTS = SUP * TSLOT
SBC = S // 128        # seg-id blocks per direction
CHUNK_ROWS = (32768, 32768, 32768, NFULL - 3 * 32768)
# cst16 (f16, replicated) columns: W_self | W1 | W2
C_WS, C_W1, C_W2 = 0, 128, 256
NC16 = 384
# cst32 (f32, per-core) columns
C_B, C_IVI, C_IVO, C_XS = 0, 1, 99, 197
NC32 = 295
A_IOTA, A_ID = 0, 128

_verbose = True
# Device f32->int8 conversion rounding: if it truncates toward zero, shift
# dequantized values to bucket centers on host (i8 + 0.5*sign(i8)).
TRUNC_COMP = os.environ.get("K2_TRUNC_COMP", "0") == "1"


def _log(tag, t0):
    if _verbose:
        now = _time.perf_counter()
        print(f"[kernel-timing] {tag}: {now - t0:.3f}s", file=sys.stderr)
    return _time.perf_counter()


# ---------------------------------------------------------------- planner ---

_KEYBASE = None


def _key_tables():
    global _KEYBASE
    if _KEYBASE is None:
        k = np.arange(P * NSUP * 4 * SUP)
        k_core = k // (NSUP * 4 * SUP)
        rem = k % (NSUP * 4 * SUP)
        k_s = rem // (4 * SUP)
        rem2 = rem % (4 * SUP)
        k_c = rem2 // SUP
        k_t = rem2 % SUP
        caps = np.asarray(CAPS_SLOT)
        base = (k_core * S + k_s * SUP_SLOTS + SUP * np.asarray(CUMCAP)[k_c]
                + k_t * caps[k_c])
        _KEYBASE = (base.astype(np.int64), caps[k_c])
    return _KEYBASE


def _plan_dir(gat, seg):
    """Slot placement for one direction. gat/seg: int32 global node ids.

    Returns (gflat int16 [P*S], sflat int8 [P*S]) or None on cap overflow.
    """
    core, lsec = np.divmod(seg, NSH)
    tloc = lsec >> 7
    lane = (lsec & 127).astype(np.int8)
    q, r = np.divmod(gat, NSH)
    row = q * NSH_PAD + r
    chunk = row >> 15
    lidx = (row & 32767).astype(np.int16)
    ssup = tloc // SUP
    tin = tloc - ssup * SUP
    key = (((core * NSUP + ssup) * 4 + chunk) * SUP + tin).astype(np.int16)
    kbase, kcap = _key_tables()
    counts = np.bincount(key, minlength=kbase.size)
    if np.any(counts > kcap):
        return None, None
    order = np.argsort(key, kind="stable")
    starts = np.concatenate(([0], np.cumsum(counts)[:-1]))
    rank = np.arange(E, dtype=np.int64) - np.repeat(starts, counts)
    slot = kbase[key[order]] + rank
    gflat = np.zeros(P * S, np.int16)   # pad gathers row 0 of its chunk
    sflat = np.full(P * S, -1, np.int8)  # pad segid -1 -> masked
    gflat[slot] = lidx[order]
    sflat[slot] = lane[order]
    return gflat, sflat


# ------------------------------------------------------------- device prog --

def _build_program(variant="full"):
    from concourse import bacc, tile, mybir, library_config

    f32 = mybir.dt.float32
    f16 = mybir.dt.float16
    i16 = mybir.dt.int16
    i8 = mybir.dt.int8
    AF = mybir.ActivationFunctionType
    OP = mybir.AluOpType
    nc = bacc.Bacc("TRN2", target_bir_lowering=False, debug=False,
                   num_swdge_queues=3, num_devices=P)

    xq = nc.dram_tensor("x_q", [NSH_PAD, D], i8, kind="ExternalInput")
    idx_i = nc.dram_tensor("idx_i", [16, S // 16], i16, kind="ExternalInput")
    idx_o = nc.dram_tensor("idx_o", [16, S // 16], i16, kind="ExternalInput")
    seg_i = nc.dram_tensor("seg_i", [128, SBC], i8, kind="ExternalInput")
    seg_o = nc.dram_tensor("seg_o", [128, SBC], i8, kind="ExternalInput")
    cst16 = nc.dram_tensor("cst16", [128, NC16], f16, kind="ExternalInput")
    cst32 = nc.dram_tensor("cst32", [128, NC32], f32, kind="ExternalInput")
    aux8 = nc.dram_tensor("aux8", [128, 256], i8, kind="ExternalInput")
    x_mine = nc.dram_tensor("x_mine", [NSH_PAD, D], f16)
    x_full = nc.dram_tensor("x_full", [NFULL, D], f16)
    out_q = nc.dram_tensor("out_q", [NSH_PAD, D], i8, kind="ExternalOutput")
    q_out = nc.dram_tensor("q_out", [128, NT], f32, kind="ExternalOutput")

    with tile.TileContext(nc) as tc:
        nc.gpsimd.load_library(library_config.mlp)
        with tc.tile_pool(name="const", bufs=1) as cp, \
             tc.tile_pool(name="eg", bufs=2) as gp, \
             tc.tile_pool(name="mp", bufs=3) as mp, \
             tc.tile_pool(name="pa", bufs=2, space="PSUM") as pa, \
             tc.tile_pool(name="pt", bufs=3, space="PSUM") as pt:
            cs = cp.tile([128, NC16], f16)
            nc.sync.dma_start(cs[:], cst16[:])
            cf = cp.tile([128, NC32], f32)
            nc.sync.dma_start(cf[:], cst32[:])
            aux_i8 = cp.tile([128, 256], i8)
            nc.sync.dma_start(aux_i8[:], aux8[:])
            ax = cp.tile([128, 256], f32)
            nc.vector.tensor_copy(ax[:], aux_i8[:])
            seg_i8 = cp.tile([128, 2 * SBC], i8)
            nc.sync.dma_start(seg_i8[:, :SBC], seg_i[:])
            nc.sync.dma_start(seg_i8[:, SBC:], seg_o[:])
            seg = cp.tile([128, 2 * SBC], f32)
            nc.vector.tensor_copy(seg[:], seg_i8[:])
            idxt = {}
            for nm, srct in (("i", idx_i), ("o", idx_o)):
                t = cp.tile([128, S // 16], i16, tag=f"idx_{nm}")
                nc.sync.dma_start(t[0:16, :], srct[:])
                nc.sync.dma_start(t[16:32, :], srct[:])
                nc.sync.dma_start(t[32:64, :], t[0:32, :])
                nc.sync.dma_start(t[64:128, :], t[0:64, :])
                idxt[nm] = t
            qsb = cp.tile([128, NT], f32, tag="qsb")

            # dequantize own shard, then allgather f16
            for t in range(NT):
                xi = mp.tile([128, D], i8, tag="xi")
                nc.sync.dma_start(xi[:], xq[t * 128:(t + 1) * 128, :])
                xc = mp.tile([128, D], f32, tag="xc")
                nc.vector.tensor_copy(xc[:], xi[:])
                xf = mp.tile([128, D], f16, tag="xf")
                nc.scalar.activation(xf[:], xc[:], AF.Copy,
                                     scale=cf[:, C_XS + t:C_XS + t + 1])
                nc.sync.dma_start(x_mine[t * 128:(t + 1) * 128, :], xf[:])
            if variant != "deq":
                nc.gpsimd.collective_compute(
                    "AllGather", OP.bypass,
                    replica_groups=[list(range(P))],
                    ins=[x_mine.ap().opt()],
                    outs=[x_full.ap().opt()],
                )
            if variant != "full":
                # debug epilogue: copy x tiles through to output
                nc.vector.memset(qsb[:], 1.0)
                srcten = x_mine if variant == "deq" else x_full
                for t in range(NT):
                    xt = mp.tile([128, D], f16, tag="dxt")
                    nc.sync.dma_start(xt[:], srcten[t * 128:(t + 1) * 128, :])
                    oi8 = mp.tile([128, D], i8, tag="doi8")
                    nc.vector.tensor_copy(oi8[:], xt[:])
                    nc.sync.dma_start(out_q[t * 128:(t + 1) * 128, :], oi8[:])
                nc.sync.dma_start(q_out[:], qsb[:])

            do_agg = variant in ("g2", "full")
            nsup_emit = NSUP if variant in ("g1", "g2", "full") else 0
            qmap = {("i", 0): 0, ("i", 1): 1, ("i", 2): 2, ("i", 3): 0,
                    ("o", 0): 1, ("o", 1): 2, ("o", 2): 0, ("o", 3): 1}
            for s in range(nsup_emit):
                et = {}
                for nm in ("i", "o"):
                    for c in range(4):
                        n_idx = SUP * CAPS_SLOT[c]
                        tt = gp.tile([128, n_idx // 128, D], f16,
                                     tag=f"eg_{nm}{c}")
                        base = s * SUP_SLOTS + SUP * CUMCAP[c]
                        src = x_full[c * 32768:c * 32768 + CHUNK_ROWS[c], :]
                        # <=896 idxs per call (SWDGE ring limit)
                        for k in range(0, n_idx, 896):
                            n_k = min(896, n_idx - k)
                            nc.gpsimd.dma_gather(
                                tt[:, k // 128:(k + n_k) // 128, :], src,
                                idxt[nm][:, (base + k) // 16:
                                          (base + k + n_k) // 16],
                                n_k, n_k, D, queue_num=qmap[(nm, c)])
                        et[(nm, c)] = tt
                for tl in range(SUP if do_agg else 0):
                    t = s * SUP + tl
                    aggs = {}
                    for d_i, nm in enumerate(("i", "o")):
                        ps = pa.tile([128, D], f32, tag=f"agg_{nm}")
                        k = 0
                        for c in range(4):
                            for b in range(NB_C[c]):
                                blk = (s * SUP_SLOTS + SUP * CUMCAP[c]
                                       + tl * CAPS_SLOT[c] + b * 128)
                                segcol = d_i * SBC + blk // 128
                                oh = mp.tile([128, D], f16, tag="oh")
                                nc.vector.tensor_scalar(
                                    oh[:], ax[:, A_IOTA:A_IOTA + 128],
                                    seg[:, segcol:segcol + 1], None,
                                    OP.is_equal)
                                eb = et[(nm, c)]
                                col = (tl * CAPS_SLOT[c]) // 128 + b
                                nc.tensor.matmul(ps[:], eb[:, col, :], oh[:],
                                                 start=(k == 0),
                                                 stop=(k == NBLK_TILE - 1))
                                k += 1
                        aggs[nm] = ps
                    if variant == "g2":
                        Ai = mp.tile([128, D], f16, tag="Ai")
                        nc.scalar.activation(Ai[:], aggs["i"][:], AF.Copy)
                        Ao = mp.tile([128, D], f16, tag="Ao")
                        nc.scalar.activation(Ao[:], aggs["o"][:], AF.Copy)
                        continue
                    # epilogue for tile t
                    xt = mp.tile([128, D], f16, tag="xt")
                    nc.sync.dma_start(xt[:], x_mine[t * 128:(t + 1) * 128, :])
                    xt32 = mp.tile([128, D], f32, tag="xt32")
                    nc.vector.tensor_copy(xt32[:], xt[:])
                    xT_ps = pt.tile([128, D], f32, tag="tp")
                    nc.tensor.matmul(xT_ps[:], xt32[:],
                                     ax[:, A_ID:A_ID + 128],
                                     start=True, stop=True, is_transpose=True)
                    xT = mp.tile([128, D], f16, tag="xTs")
                    nc.vector.tensor_copy(xT[:], xT_ps[:])
                    Ai = mp.tile([128, D], f16, tag="Ai")
                    nc.scalar.activation(Ai[:], aggs["i"][:], AF.Copy)
                    Ao = mp.tile([128, D], f16, tag="Ao")
                    nc.scalar.activation(Ao[:], aggs["o"][:], AF.Copy)
                    yTs = pt.tile([128, D], f32, tag="tp")
                    nc.tensor.matmul(yTs[:], cs[:, C_WS:C_WS + 128], xT[:],
                                     start=True, stop=True)
                    yTi = pt.tile([128, D], f32, tag="tp")
                    nc.tensor.matmul(yTi[:], cs[:, C_W1:C_W1 + 128], Ai[:],
                                     start=True, stop=True)
                    yTo = pt.tile([128, D], f32, tag="tp")
                    nc.tensor.matmul(yTo[:], cs[:, C_W2:C_W2 + 128], Ao[:],
                                     start=True, stop=True)
                    cself = mp.tile([128, D], f32, tag="cself")
                    nc.vector.tensor_scalar(cself[:], yTs[:],
                                            cf[:, C_B:C_B + 1], None, OP.add)
                    ci = mp.tile([128, D], f32, tag="ci")
                    nc.scalar.activation(ci[:], yTi[:], AF.Copy)
                    co = mp.tile([128, D], f32, tag="co")
                    nc.scalar.activation(co[:], yTo[:], AF.Copy)
                    y_self = pt.tile([128, D], f32, tag="tp")
                    nc.tensor.matmul(y_self[:], cself[:],
                                     ax[:, A_ID:A_ID + 128],
                                     start=True, stop=True, is_transpose=True)
                    y_in = pt.tile([128, D], f32, tag="tp")
                    nc.tensor.matmul(y_in[:], ci[:],
                                     ax[:, A_ID:A_ID + 128],
                                     start=True, stop=True, is_transpose=True)
                    y_out = pt.tile([128, D], f32, tag="tp")
                    nc.tensor.matmul(y_out[:], co[:],
                                     ax[:, A_ID:A_ID + 128],
                                     start=True, stop=True, is_transpose=True)
                    ysb = mp.tile([128, D], f32, tag="ysb")
                    nc.scalar.activation(ysb[:], y_self[:], AF.Copy)
                    ysum = mp.tile([128, D], f32, tag="ysum")
                    nc.vector.scalar_tensor_tensor(
                        ysum[:], y_in[:], cf[:, C_IVI + t:C_IVI + t + 1],
                        ysb[:], OP.mult, OP.add)
                    y2 = mp.tile([128, D], f32, tag="y2")
                    nc.vector.scalar_tensor_tensor(
                        y2[:], y_out[:], cf[:, C_IVO + t:C_IVO + t + 1],
                        ysum[:], OP.mult, OP.add)
                    rmax = mp.tile([128, 1], f32, tag="rmax")
                    nc.vector.tensor_reduce(rmax[:], y2[:],
                                            mybir.AxisListType.X, OP.max,
                                            apply_absolute_value=True)
                    rs = mp.tile([128, 1], f32, tag="rs")
                    nc.scalar.activation(rs[:], rmax[:], AF.Copy,
                                         scale=1.0 / 126.0)
                    nc.vector.reciprocal(qsb[:, t:t + 1], rs[:])
                    oi8 = mp.tile([128, D], i8, tag="oi8")
                    nc.vector.tensor_scalar(oi8[:], y2[:], qsb[:, t:t + 1],
                                            None, OP.mult)
                    nc.sync.dma_start(out_q[t * 128:(t + 1) * 128, :], oi8[:])
            nc.sync.dma_start(q_out[:], qsb[:])

    nc.compile()
    return nc


# --------------------------------------------------------------- AOT setup --

_AOT = {}


def _aot_compile():
    t0 = _time.perf_counter()
    import jax
    from jax.sharding import Mesh, PartitionSpec, NamedSharding
    from jax.experimental.shard_map import shard_map
    from concourse import bass2jax, mybir

    nc = _build_program(os.environ.get("K2_VARIANT", "full"))
    t0 = _log("aot: build+bass-compile", t0)

    bass2jax.install_neuronx_cc_hook()
    partition_name = (nc.partition_id_tensor.name
                      if nc.partition_id_tensor else None)
    in_names, out_names, out_avals, zero_outs = [], [], [], []
    for alloc in nc.m.functions[0].allocations:
        if not isinstance(alloc, mybir.MemoryLocationSet):
            continue
        name = alloc.memorylocations[0].name
        if alloc.kind == "ExternalInput":
            if name != partition_name:
                in_names.append(name)
        elif alloc.kind == "ExternalOutput":
            shape = tuple(alloc.tensor_shape)
            dtype = mybir.dt.np(alloc.dtype)
            out_names.append(name)
            out_avals.append(jax.core.ShapedArray(shape, dtype))
            zero_outs.append(np.zeros(shape, dtype))
    n_params = len(in_names)
    n_outs = len(out_avals)
    in_names_full = (in_names + out_names
                     + ([partition_name] if partition_name else []))

    def _body(*args):
        operands = list(args)
        if partition_name is not None:
            operands.append(bass2jax.partition_id_tensor())
        outs = bass2jax._bass_exec_p.bind(
            *operands,
            out_avals=tuple(out_avals),
            in_names=tuple(in_names_full),
            out_names=tuple(out_names),
            lowering_input_output_aliases=(),
            sim_require_finite=True,
            sim_require_nnan=True,
            nc=nc,
        )
        return tuple(outs)

    devices = jax.devices()[:P]
    mesh = Mesh(np.asarray(devices), ("core",))
    in_specs = (PartitionSpec("core"),) * (n_params + n_outs)
    out_specs = (PartitionSpec("core"),) * n_outs
    donate = tuple(range(n_params, n_params + n_outs))
    sharded = jax.jit(
        shard_map(_body, mesh=mesh, in_specs=in_specs, out_specs=out_specs,
                  check_rep=False),
        donate_argnums=donate, keep_unused=True)

    shape_of = {}
    for alloc in nc.m.functions[0].allocations:
        if (isinstance(alloc, mybir.MemoryLocationSet)
                and alloc.kind == "ExternalInput"):
            shape_of[alloc.memorylocations[0].name] = (
                tuple(alloc.tensor_shape), mybir.dt.np(alloc.dtype))
    abstract = []
    for name in in_names:
        shp, dt = shape_of[name]
        abstract.append(jax.ShapeDtypeStruct((P * shp[0],) + shp[1:], dt))
    for z in zero_outs:
        abstract.append(
            jax.ShapeDtypeStruct((P * z.shape[0],) + z.shape[1:], z.dtype))
    compiled = sharded.lower(*abstract).compile()
    t0 = _log("aot: neff-compile", t0)

    sharding = NamedSharding(mesh, PartitionSpec("core"))
    _AOT.update(dict(
        nc=nc, compiled=compiled, in_names=in_names, out_names=out_names,
        zero_outs=zero_outs, sharding=sharding, dz=None, jax=jax))
    _make_zeros()
    _log("aot: zero-put", t0)


def _make_zeros():
    jax = _AOT["jax"]
    _AOT["dz"] = [
        jax.device_put(
            np.zeros((P * z.shape[0],) + z.shape[1:], z.dtype),
            _AOT["sharding"])
        for z in _AOT["zero_outs"]]
    jax.block_until_ready(_AOT["dz"])


if not os.environ.get("KERNEL_NO_AOT"):
    try:
        _aot_compile()
    except Exception as _e:  # pragma: no cover
        print(f"[kernel] AOT compile failed ({_e!r}); will compile lazily",
              file=sys.stderr)
        _AOT.clear()


# ------------------------------------------------------------------ kernel --

def kernel(x, W_self, b_self, W_s2d, b_s2d, W_d2s, b_d2s, edge_index):
    t0 = _time.perf_counter()
    x = np.ascontiguousarray(x, np.float32)
    src = np.asarray(edge_index[0], np.int64).astype(np.int32)
    dst = np.asarray(edge_index[1], np.int64).astype(np.int32)

    if not _AOT:
        _aot_compile()
    jax = _AOT["jax"]
    sharding = _AOT["sharding"]

    # --- quantize x (per-row) and start its upload first ---
    rmax = np.abs(x).max(axis=1)
    np.maximum(rmax, 1e-6, out=rmax)
    xs16 = (rmax * (1.0 / 126.0)).astype(np.float16)   # x ~= i8 * xs16
    inv = 1.0 / xs16.astype(np.float32)
    buf = np.multiply(x, inv[:, None])
    np.rint(buf, out=buf)
    xq_pad = np.zeros((P, NSH_PAD, D), np.int8)
    xq_pad[:, :NSH] = buf.reshape(P, NSH, D)
    dev_xq = jax.device_put(xq_pad.reshape(P * NSH_PAD, D), sharding)

    # --- consts (cheap) and their upload ---
    W_self = np.asarray(W_self, np.float32)
    b_tot = (np.asarray(b_self, np.float32)
             + (1.0 - ALPHA) * np.asarray(b_s2d, np.float32)
             + ALPHA * np.asarray(b_d2s, np.float32))
    c16 = np.zeros((128, NC16), np.float16)
    c16[:, C_WS:C_WS + 128] = W_self.astype(np.float16)
    c16[:, C_W1:C_W1 + 128] = ((1.0 - ALPHA)
                               * np.asarray(W_s2d, np.float32)).astype(np.float16)
    c16[:, C_W2:C_W2 + 128] = (ALPHA
                               * np.asarray(W_d2s, np.float32)).astype(np.float16)
    c16r = np.broadcast_to(c16, (P, 128, NC16)).reshape(P * 128, NC16)
    dev_c16 = jax.device_put(np.ascontiguousarray(c16r), sharding)

    deg_in = np.bincount(dst, minlength=N).astype(np.float32)
    deg_out = np.bincount(src, minlength=N).astype(np.float32)
    c32 = np.zeros((P, 128, NC32), np.float32)
    c32[:, :, C_B] = b_tot
    for col, v in ((C_IVI, 1.0 / np.maximum(deg_in, 1.0)),
                   (C_IVO, 1.0 / np.maximum(deg_out, 1.0))):
        a = np.ones((P, NSH_PAD), np.float32)
        a[:, :NSH] = v.reshape(P, NSH)
        c32[:, :, col:col + NT] = a.reshape(P, NT, 128).transpose(0, 2, 1)
    sa = np.zeros((P, NSH_PAD), np.float32)
    sa[:, :NSH] = xs16.astype(np.float32).reshape(P, NSH)
    c32[:, :, C_XS:C_XS + NT] = sa.reshape(P, NT, 128).transpose(0, 2, 1)
    dev_c32 = jax.device_put(c32.reshape(P * 128, NC32), sharding)
    a8 = np.zeros((128, 256), np.int8)
    a8[:, A_IOTA:A_IOTA + 128] = np.arange(128, dtype=np.int8)
    a8[:, A_ID:A_ID + 128] = np.eye(128, dtype=np.int8)
    dev_a8 = jax.device_put(
        np.broadcast_to(a8, (P, 128, 256)).reshape(P * 128, 256), sharding)
    t0 = _log("quant+consts+put", t0)

    # --- plan per direction, uploading each as soon as it is ready ---
    gi, si = _plan_dir(src, dst)
    if gi is None:
        raise RuntimeError("kernel2: slot capacity exceeded")
    dev_ii = jax.device_put(
        np.ascontiguousarray(gi.reshape(P, S // 16, 16).transpose(0, 2, 1))
        .reshape(P * 16, S // 16), sharding)
    dev_si = jax.device_put(
        np.ascontiguousarray(si.reshape(P, SBC, 128).transpose(0, 2, 1))
        .reshape(P * 128, SBC), sharding)
    go, so = _plan_dir(dst, src)
    if go is None:
        raise RuntimeError("kernel2: slot capacity exceeded")
    dev_io = jax.device_put(
        np.ascontiguousarray(go.reshape(P, S // 16, 16).transpose(0, 2, 1))
        .reshape(P * 16, S // 16), sharding)
    dev_so = jax.device_put(
        np.ascontiguousarray(so.reshape(P, SBC, 128).transpose(0, 2, 1))
        .reshape(P * 128, SBC), sharding)
    t0 = _log("plan+pack+put", t0)

    by_name = {"x_q": dev_xq, "idx_i": dev_ii, "idx_o": dev_io,
               "seg_i": dev_si, "seg_o": dev_so,
               "cst16": dev_c16, "cst32": dev_c32, "aux8": dev_a8}
    dev_args = [by_name[name] for name in _AOT["in_names"]]
    _AOT["_dbg_args"] = dev_args
    if _AOT["dz"] is None:
        _make_zeros()
    dz = _AOT["dz"]
    _AOT["dz"] = None
    compiled = _AOT["compiled"]
    outs = compiled(*dev_args, *dz)
    t0 = _log("exec-dispatch", t0)

    try:
        out_by = dict(zip(_AOT["out_names"], outs))
        for nm in ("q_out", "out_q"):
            try:
                out_by[nm].copy_to_host_async()
            except Exception:
                pass
        qh = np.asarray(out_by["q_out"]).reshape(P, 128, NT)
        i8 = np.asarray(out_by["out_q"]).reshape(P, NSH_PAD, D)
    except Exception as e:  # transient device failure: one retry
        print(f"[kernel] exec failed ({e!r}); retrying once", file=sys.stderr)
        _make_zeros()
        dz = _AOT["dz"]
        _AOT["dz"] = None
        outs = compiled(*dev_args, *dz)
        out_by = dict(zip(_AOT["out_names"], outs))
        qh = np.asarray(out_by["q_out"]).reshape(P, 128, NT)
        i8 = np.asarray(out_by["out_q"]).reshape(P, NSH_PAD, D)
    t0 = _log("exec+fetch", t0)
    # scale per node: q[core, lane, tile] -> node core*NSH + tile*128 + lane
    sc = (1.0 / qh.astype(np.float64)).astype(np.float32)
    scv = sc.transpose(0, 2, 1).reshape(P, NSH_PAD)[:, :NSH]
    a = i8[:, :NSH].astype(np.float32)
    if TRUNC_COMP:
        a += np.sign(a) * 0.5
    res = a * scv[:, :, None]
    _log("dequant", t0)
    return res.reshape(N, D)
